# revision 12
# baseline (speedup 1.0000x reference)
"""Fused PVT-style transformer block kernel for Trainium2 (8 NeuronCores).

Sharding: pure data-parallel over batch B=8 -> one batch item per core.
Layout: channel-major ("transposed") activations [C(part), N(free)] throughout;
host pre-transposes x and relative_pos, post-transposes the output.

Per-core pipeline (N=3136=56x56 tokens, C=256, 4 heads x 64, KV=784=28x28,
HID=1024):
  LN1 (PE ones-matmul stats + PE K=1 broadcast + DVE apply; gamma/beta folded
  into downstream weights) -> q/k/v projections (bf16 PE) with the 2x2/s2
  spatial-reduction depthwise conv on DVE -> flash attention per (head,
  q-tile): scores^T = k^T.T @ q^T with rel-pos bias added via identity matmul
  into PSUM, exp on ACT (no max-subtraction: logits are O(1)), AV matmul with
  ones-row-augmented V giving the softmax denominator for free -> wo
  projection + residual -> LN2 -> conv1x1 (+gelu+bn1) -> 3x3 depthwise conv
  split across PE (fp32r diagonal matmuls into PSUM) and DVE (fused
  scalar_tensor_tensor taps) -> gelu -> conv1x1 (bn2/pbn folded) + residual ->
  final 3x3 depthwise conv (residual folded into center tap) -> output.
"""

import numpy as np
import ml_dtypes

B, N, C, NH, DH, KV, HID = 8, 3136, 256, 4, 64, 784, 1024
HS = WS = 56
NT = 448            # n-tile (8 rows of 56)
NNT = N // NT       # 7
KT = 112            # kv tile
NKT = KV // KT      # 7
EPS = 1e-5
BF16 = ml_dtypes.bfloat16

DW_PE_TILES = (0, 1, 2, 3, 4, 5)   # HID ch-tiles whose dwconv runs on PE
BLK_PE_TILES = (0,)             # blk dwconv ch-tiles on PE

TAPS = [(dy, dx) for dy in (-1, 0, 1) for dx in (-1, 0, 1) if (dy, dx) != (0, 0)]


def tap_idx(dy, dx):
    return (dy + 1) * 3 + (dx + 1)


def _build_program():
    import concourse.bacc as bacc
    import concourse.mybir as mybir
    import concourse.tile as tile
    from contextlib import ExitStack

    dt = mybir.dt
    F32, BF, F32R = dt.float32, dt.bfloat16, dt.float32r
    Alu = mybir.AluOpType
    Act = mybir.ActivationFunctionType

    nc = bacc.Bacc("TRN2", target_bir_lowering=False, debug=False, num_devices=8)

    def din(name, shape, dtype):
        return nc.dram_tensor(name, shape, dtype, kind="ExternalInput")

    xT_d = din("xT", [C, N], F32)
    rpT_d = din("rpT", [NH, KV, N], BF)
    wqT_d = din("wqT", [C, C], BF)
    wkT_d = din("wkT", [C, C], BF)
    wvT_d = din("wvT", [C, C], BF)
    woT_d = din("woT", [C, C], BF)
    w1T_d = din("w1T", [C, HID], BF)
    w2T_d = din("w2T", [HID, C], BF)
    bq_d = din("bq", [C], F32)
    bk_d = din("bk", [C], F32)
    bvr_d = din("bvr", [1, C], F32)
    bo_d = din("bo", [C], F32)
    b1_d = din("b1", [HID], F32)
    a1_d = din("a1", [HID], F32)
    c1_d = din("c1", [HID], F32)
    b2r_d = din("b2r", [C], F32)
    srw_d = din("srw", [C, 4], F32)
    dw9_d = din("dw9", [HID, 9], F32)
    dwb_d = din("dwb", [HID], F32)
    bk9_d = din("bk9", [C, 9], F32)
    bkb_d = din("bkb", [1, C], BF)
    eyeb_d = din("eyeb", [128, 128], BF)
    eyef_d = din("eyef", [128, 128], F32)
    onesr_d = din("onesr", [1, 128], BF)
    fT_d = nc.dram_tensor("fT", [C, N], F32, kind="ExternalOutput")

    def r32(ap):
        return ap.bitcast(F32R)

    with tile.TileContext(nc) as tc, ExitStack() as octx:
        wpool = octx.enter_context(tc.tile_pool(name="weights", bufs=1))
        persist = octx.enter_context(tc.tile_pool(name="persist", bufs=1))

        def wload(dram_ap, shape, dtype, tag):
            t = wpool.tile(shape, dtype, tag=tag, name=tag)
            nc.sync.dma_start(out=t[:], in_=dram_ap)
            return t

        wq_sb = [wload(wqT_d[k * 128:(k + 1) * 128, :], [128, C], BF, f"wq{k}") for k in range(2)]
        wk_sb = [wload(wkT_d[k * 128:(k + 1) * 128, :], [128, C], BF, f"wk{k}") for k in range(2)]
        wv_sb = [wload(wvT_d[k * 128:(k + 1) * 128, :], [128, C], BF, f"wv{k}") for k in range(2)]
        wo_sb = [wload(woT_d[k * 128:(k + 1) * 128, :], [128, C], BF, f"wo{k}") for k in range(2)]
        w1_sb = [wload(w1T_d[k * 128:(k + 1) * 128, :], [128, HID], BF, f"w1{k}") for k in range(2)]
        w2_sb = [wload(w2T_d[k * 128:(k + 1) * 128, :], [128, C], BF, f"w2{k}") for k in range(8)]
        eyeb = wload(eyeb_d[:, :], [128, 128], BF, "eyeb")
        eyef = wload(eyef_d[:, :], [128, 128], F32, "eyef")
        onesr = wload(onesr_d[:, :], [1, 128], BF, "onesr")
        srw_sb = [wload(srw_d.ap().rearrange("(t p) k -> p t k", p=128)[:, t, :],
                        [128, 4], F32, f"srw{t}") for t in range(2)]
        dw9_sb = [wload(dw9_d.ap().rearrange("(t p) k -> p t k", p=128)[:, t, :],
                        [128, 9], F32, f"dw9_{t}") for t in range(8)]
        bk9_sb = [wload(bk9_d.ap().rearrange("(t p) k -> p t k", p=128)[:, t, :],
                        [128, 9], F32, f"bk9_{t}") for t in range(2)]

        def vload(dram, n, tag):
            t = wpool.tile([128, n // 128], F32, tag=tag, name=tag)
            nc.sync.dma_start(out=t[:], in_=dram.ap().rearrange("(t p) -> p t", p=128))
            return t

        bq_sb = vload(bq_d, C, "bq")
        bk_sb = vload(bk_d, C, "bk")
        bo_sb = vload(bo_d, C, "bo")
        b1_sb = vload(b1_d, HID, "b1")
        a1_sb = vload(a1_d, HID, "a1")
        c1_sb = vload(c1_d, HID, "c1")
        b2r_sb = vload(b2r_d, C, "b2r")
        dwb_sb = vload(dwb_d, HID, "dwb")
        bvr_bc = wpool.tile([128, C], F32, tag="bvr")
        nc.sync.dma_start(out=bvr_bc[:], in_=bvr_d.ap().partition_broadcast(128))
        bkb_row = wload(bkb_d[:, :], [1, C], BF, "bkb")
        onesn = wpool.tile([1, NT], BF, tag="onesn")
        nc.vector.memset(onesn[:], 1.0)
        onescol = wpool.tile([128, 1], F32, tag="onescol")
        nc.vector.memset(onescol[:], 1.0)
        onescol_b = wpool.tile([128, 1], BF, tag="onescol_b")
        nc.vector.memset(onescol_b[:], 1.0)
        epscol = wpool.tile([128, 1], F32, tag="epscol")
        nc.vector.memset(epscol[:], EPS)

        # residual stream (f32), reused x -> x+attn -> x+mlp in place
        xres = [persist.tile([128, N], F32, tag=f"xres{t}", name=f"xres{t}") for t in range(2)]
        for t in range(2):
            for nt in range(NNT):
                cs = slice(nt * NT, (nt + 1) * NT)
                nc.sync.dma_start(out=xres[t][:, cs], in_=xT_d[t * 128:(t + 1) * 128, cs])
        # LN output (bf16, un-affine'd), reused for LN1 and LN2
        hbuf = [persist.tile([128, N], BF, tag=f"hbuf{t}", name=f"hbuf{t}") for t in range(2)]

        def layer_norm(suffix):
            """hbuf <- (xres - mean_c) * rsqrt(var_c + eps)."""
            with ExitStack() as ctx:
                sqp = ctx.enter_context(tc.tile_pool(name=f"ln_sq{suffix}", bufs=3))
                stp = ctx.enter_context(tc.tile_pool(name=f"ln_st{suffix}", bufs=2, space="PSUM"))
                bcp = ctx.enter_context(tc.tile_pool(name=f"ln_bc{suffix}", bufs=2, space="PSUM"))
                rowp = ctx.enter_context(tc.tile_pool(name=f"ln_row{suffix}", bufs=2))
                tmpp = ctx.enter_context(tc.tile_pool(name=f"ln_tmp{suffix}", bufs=3))
                for nt in range(NNT):
                    cs = slice(nt * NT, (nt + 1) * NT)
                    st0 = stp.tile([1, NT], F32, tag="st0")
                    for ct in range(2):
                        nc.tensor.matmul(st0[:], onescol[:], xres[ct][:, cs],
                                         start=(ct == 0), stop=(ct == 1))
                    st1 = stp.tile([1, NT], F32, tag="st1")
                    for ct in range(2):
                        sq = sqp.tile([128, NT], BF)
                        nc.scalar.square(sq[:], xres[ct][:, cs])
                        nc.tensor.matmul(st1[:], onescol_b[:], sq[:],
                                         start=(ct == 0), stop=(ct == 1))
                    m2 = rowp.tile([1, NT], F32, tag="m2")
                    nc.scalar.activation(m2[:], st0[:], Act.Square, scale=1.0 / C)
                    var = rowp.tile([1, NT], F32, tag="var")
                    nc.vector.scalar_tensor_tensor(var[:], st1[:], 1.0 / C, m2[:],
                                                   op0=Alu.mult, op1=Alu.subtract)
                    sd = rowp.tile([1, NT], F32, tag="sd")
                    nc.scalar.activation(sd[:], var[:], Act.Sqrt, bias=epscol[0:1, :])
                    arow = rowp.tile([1, NT], BF, tag="arow")
                    with nc.allow_low_precision("bf16 rstd broadcast row"):
                        nc.vector.reciprocal(arow[:], sd[:])
                    crow = rowp.tile([1, NT], BF, tag="crow")
                    nc.vector.scalar_tensor_tensor(crow[:], st0[:], -1.0 / C, arow[:],
                                                   op0=Alu.mult, op1=Alu.mult)
                    aps = bcp.tile([128, NT], F32, tag="abc")
                    nc.tensor.matmul(aps[:], onesr[:], arow[:])
                    cps = bcp.tile([128, NT], F32, tag="abc")
                    nc.tensor.matmul(cps[:], onesr[:], crow[:])
                    for ct in range(2):
                        t0 = tmpp.tile([128, NT], F32)
                        nc.vector.tensor_mul(t0[:], xres[ct][:, cs], aps[:])
                        nc.vector.tensor_add(hbuf[ct][:, cs], t0[:], cps[:])

        # ================= stage 1: LN1 + attention =================
        with ExitStack() as ctx:
            layer_norm("1")
            apool = ctx.enter_context(tc.tile_pool(name="attn_sb", bufs=1))
            cT = [apool.tile([128, KV], BF, tag=f"cT{t}", name=f"cT{t}") for t in range(2)]
            k_sb = [apool.tile([128, KV], BF, tag=f"k{t}", name=f"k{t}") for t in range(2)]
            v_sb = apool.tile([128, NKT * 260], BF, tag="v", name="v_sb")
            q_sb = [apool.tile([128, N], BF, tag=f"q{t}", name=f"q{t}") for t in range(2)]
            o_cat = [apool.tile([128, N], BF, tag=f"ocat{t}", name=f"ocat{t}") for t in range(2)]

            with ExitStack() as pctx:
                mmp = pctx.enter_context(tc.tile_pool(name="proj_ps", bufs=3, space="PSUM"))
                # SR 2x2/s2 depthwise conv on hbuf -> cT [C, 784] bf16
                for ct in range(2):
                    h4 = hbuf[ct][:].rearrange("p (h a w b) -> p h a w b", a=2, b=2, h=28, w=28)
                    c3 = cT[ct][:].rearrange("p (h w) -> p h w", w=28)
                    nc.vector.tensor_scalar_mul(c3[:, :, :], h4[:, :, 0, :, 0], srw_sb[ct][:, 0:1])
                    for ky, kx in ((0, 1), (1, 0), (1, 1)):
                        ti = ky * 2 + kx
                        nc.vector.scalar_tensor_tensor(
                            c3[:, :, :], h4[:, :, ky, :, kx], srw_sb[ct][:, ti:ti + 1],
                            c3[:, :, :], op0=Alu.mult, op1=Alu.add)

                # k^T = wk @ cT + bk -> [256, 784] bf16
                for mt in range(2):
                    for n0, nsz in ((0, 448), (448, 336)):
                        ps = mmp.tile([128, NT], F32, tag="mm")
                        for kt in range(2):
                            nc.tensor.matmul(ps[:, :nsz], wk_sb[kt][:, mt * 128:(mt + 1) * 128],
                                             cT[kt][:, n0:n0 + nsz], start=(kt == 0), stop=(kt == 1))
                        nc.scalar.activation(k_sb[mt][:, n0:n0 + nsz], ps[:, :nsz], Act.Identity,
                                             bias=bk_sb[:, mt:mt + 1])

                # v (+ones col per head) -> v_sb [112, 7*260] bf16
                for kt in range(NKT):
                    ps = mmp.tile([128, NT], F32, tag="mm")
                    for ct in range(2):
                        nc.tensor.matmul(ps[0:KT, 0:C], cT[ct][:, kt * KT:(kt + 1) * KT],
                                         wv_sb[ct][:], start=(ct == 0), stop=(ct == 1))
                    for h in range(NH):
                        nc.vector.scalar_tensor_tensor(
                            v_sb[0:KT, kt * 260 + h * 65: kt * 260 + h * 65 + 64],
                            ps[0:KT, h * 64:(h + 1) * 64], 1.0,
                            bvr_bc[0:KT, h * 64:(h + 1) * 64], op0=Alu.mult, op1=Alu.add)
                        nc.vector.memset(v_sb[0:KT, kt * 260 + h * 65 + 64: kt * 260 + h * 65 + 65], 1.0)

                # q^T = wq @ hbuf + bq -> [256, 3136] bf16
                for mt in range(2):
                    for nt in range(NNT):
                        cs = slice(nt * NT, (nt + 1) * NT)
                        ps = mmp.tile([128, NT], F32, tag="mm")
                        for kt in range(2):
                            nc.tensor.matmul(ps[:], wq_sb[kt][:, mt * 128:(mt + 1) * 128],
                                             hbuf[kt][:, cs], start=(kt == 0), stop=(kt == 1))
                        nc.scalar.activation(q_sb[mt][:, cs], ps[:], Act.Identity,
                                             bias=bq_sb[:, mt:mt + 1])

            # flash attention (heads interleaved for PE row-group packing;
            # rel-pos bias applied as exp(s)*exp(rp) with host-precomputed
            # exp(rp) multiplied in on DVE)
            with ExitStack() as pctx:
                rpp = pctx.enter_context(tc.tile_pool(name="rp", bufs=2))
                ppp = pctx.enter_context(tc.tile_pool(name="pexp", bufs=2))
                sps = pctx.enter_context(tc.tile_pool(name="spsum", bufs=3, space="PSUM"))
                ops = pctx.enter_context(tc.tile_pool(name="opsum", bufs=2, space="PSUM"))
                rps = pctx.enter_context(tc.tile_pool(name="rpsum", bufs=1, space="PSUM"))
                rsp = pctx.enter_context(tc.tile_pool(name="rsb", bufs=2))
                for qt in range(NNT):
                    cs = slice(qt * NT, (qt + 1) * NT)
                    for h in range(NH):
                        ht, hr = h // 2, (h % 2) * 64
                        rp_t = rpp.tile([KT, NKT, NT], BF, name="rp_t")
                        nc.sync.dma_start(
                            out=rp_t[:],
                            in_=rpT_d.ap()[h].rearrange("(t p) n -> p t n", p=KT)[:, :, cs])
                        p_t = ppp.tile([KT, NKT, NT], BF, name="p_t")
                        for kt in range(NKT):
                            s_ps = sps.tile([KT, NT], F32, name="s_ps")
                            nc.tensor.matmul(s_ps[:], k_sb[ht][hr:hr + 64, kt * KT:(kt + 1) * KT],
                                             q_sb[ht][hr:hr + 64, cs], start=True, stop=True)
                            et = rsp.tile([KT, NT], BF, tag="et", name="et", bufs=3)
                            nc.scalar.activation(et[:], s_ps[:], Act.Exp)
                            nc.vector.tensor_mul(p_t[:, kt, :], et[:], rp_t[:, kt, :])
                        o_ps = ops.tile([65, NT], F32, name="o_ps")
                        for kt in range(NKT):
                            nc.tensor.matmul(o_ps[:],
                                             v_sb[0:KT, kt * 260 + h * 65: kt * 260 + (h + 1) * 65],
                                             p_t[:, kt, :], start=(kt == 0), stop=(kt == NKT - 1))
                        rrow = rsp.tile([1, NT], BF, tag="rrow", name="rrow")
                        with nc.allow_low_precision("bf16 softmax denom row"):
                            nc.vector.reciprocal(rrow[:], o_ps[64:65, :])
                        rb_ps = rps.tile([64, NT], F32, name="rb_ps")
                        nc.tensor.matmul(rb_ps[:], onesr[0:1, 0:64], rrow[:])
                        rb_sb = rsp.tile([64, NT], F32, tag="rbsb", name="rb_sb")
                        nc.scalar.activation(rb_sb[:], rb_ps[:], Act.Copy)
                        nc.vector.tensor_mul(o_cat[ht][hr:hr + 64, cs], o_ps[0:64, :], rb_sb[:])

            # wo projection + residual into xres (in place)
            with ExitStack() as pctx:
                mmp = pctx.enter_context(tc.tile_pool(name="wo_ps", bufs=3, space="PSUM"))
                for mt in range(2):
                    for nt in range(NNT):
                        cs = slice(nt * NT, (nt + 1) * NT)
                        ps = mmp.tile([128, NT], F32, tag="mm")
                        for kt in range(2):
                            nc.tensor.matmul(ps[:], wo_sb[kt][:, mt * 128:(mt + 1) * 128],
                                             o_cat[kt][:, cs], start=(kt == 0), stop=(kt == 1))
                        nc.vector.scalar_tensor_tensor(xres[mt][:, cs], ps[:], bo_sb[:, mt:mt + 1],
                                                       xres[mt][:, cs], op0=Alu.add, op1=Alu.add)

        # ================= stage 2: LN2 + conv-MLP + blk dwconv =================
        # dwconv inputs are x-padded to width 58 (zero cols 0 and 57) so all
        # taps are full-width and matmul outputs stay flat 2D.
        WP = WS + 2
        with ExitStack() as ctx:
            layer_norm("2")
            mpool = ctx.enter_context(tc.tile_pool(name="mlp_ps", bufs=3, space="PSUM"))
            dps = ctx.enter_context(tc.tile_pool(name="dw_ps", bufs=2, space="PSUM"))
            upool = ctx.enter_context(tc.tile_pool(name="u", bufs=2))
            accp = ctx.enter_context(tc.tile_pool(name="dwacc", bufs=2))
            digp = ctx.enter_context(tc.tile_pool(name="diag", bufs=2))
            y2p = ctx.enter_context(tc.tile_pool(name="y2", bufs=1))
            y2 = [y2p.tile([128, N], BF, tag=f"y2_{m}", name=f"y2_{m}") for m in range(8)]
            x3p = [y2p.tile([128, HS * WP], F32, tag=f"x3p{t}", name=f"x3p{t}")
                   for t in range(2)]
            x3b = [y2p.tile([128, HS * WP], BF, tag=f"x3b{t}", name=f"x3b{t}")
                   for t in range(2)]

            def build_diag(w9_sb):
                diag = []
                for t in range(9):
                    dg = digp.tile([128, 128], BF, tag=f"dg{t}", name=f"dg{t}")
                    nc.vector.tensor_scalar_mul(dg[:], eyeb[:], w9_sb[:, t:t + 1])
                    diag.append(dg)
                return diag

            def tap_windows(r0):
                wins = []
                for dy, dx in TAPS:
                    rlo = max(r0, 1 if dy < 0 else 0)
                    rhi = min(r0 + 8, HS - (1 if dy > 0 else 0))
                    if rlo < rhi:
                        wins.append((dy, dx, rlo, rhi))
                return wins

            def dw_pe(src3, w9_sb, bias_col, dst):
                """3x3 depthwise conv of padded bf16 src3 [128,56,58] via PE
                diag matmuls (all 9 taps); gelu evict with bias -> dst bf16."""
                diag = build_diag(w9_sb)
                for nt in range(NNT):
                    ps = dps.tile([128, NT], F32, name="dwps")
                    r0 = nt * 8
                    nc.tensor.matmul(ps[:], diag[4][:], src3[:, r0:r0 + 8, 1:57],
                                     start=True, stop=False)
                    wins = tap_windows(r0)
                    for i, (dy, dx, rlo, rhi) in enumerate(wins):
                        nc.tensor.matmul(
                            ps[:, (rlo - r0) * WS:(rhi - r0) * WS],
                            diag[tap_idx(dy, dx)][:],
                            src3[:, rlo + dy:rhi + dy, 1 + dx:57 + dx],
                            start=False, stop=(i == len(wins) - 1))
                    nc.scalar.activation(dst[:, r0 * WS:(r0 + 8) * WS], ps[:], Act.Gelu,
                                         bias=bias_col)

            def dw_dve(src3, w9_sb, bias_col, dst):
                """3x3 depthwise conv on DVE: center-tap init (+bias), 8 stt taps."""
                acc = accp.tile([128, N], F32, name="acc")
                a3 = acc[:].rearrange("p (h w) -> p h w", w=WS)
                nc.vector.tensor_scalar(a3[:, :, :], src3[:, :, 1:57], w9_sb[:, 4:5],
                                        bias_col, op0=Alu.mult, op1=Alu.add)
                for dy, dx in TAPS:
                    rlo = 1 if dy < 0 else 0
                    rhi = HS - (1 if dy > 0 else 0)
                    t = tap_idx(dy, dx)
                    nc.vector.scalar_tensor_tensor(
                        a3[:, rlo:rhi, :], src3[:, rlo + dy:rhi + dy, 1 + dx:57 + dx],
                        w9_sb[:, t:t + 1], a3[:, rlo:rhi, :], op0=Alu.mult, op1=Alu.add)
                nc.scalar.activation(dst[:], acc[:], Act.Gelu)

            for m in range(8):
                u = upool.tile([128, HS * WP], BF, name="u")
                u3 = u[:].rearrange("p (h w) -> p h w", w=WP)
                nc.vector.memset(u3[:, :, 0:1], 0.0)
                nc.vector.memset(u3[:, :, 57:58], 0.0)
                for nt in range(NNT):
                    cs = slice(nt * NT, (nt + 1) * NT)
                    ps = mpool.tile([128, NT], F32, tag="mm", name="mmps")
                    for kt in range(2):
                        nc.tensor.matmul(ps[:], w1_sb[kt][:, m * 128:(m + 1) * 128],
                                         hbuf[kt][:, cs], start=(kt == 0), stop=(kt == 1))
                    nc.scalar.activation(u3[:, nt * 8:(nt + 1) * 8, 1:57], ps[:], Act.Gelu,
                                         bias=b1_sb[:, m:m + 1])
                nc.vector.tensor_scalar(u3[:, :, 1:57], u3[:, :, 1:57], a1_sb[:, m:m + 1],
                                        c1_sb[:, m:m + 1], op0=Alu.mult, op1=Alu.add)
                if m in DW_PE_TILES:
                    dw_pe(u3, dw9_sb[m], dwb_sb[:, m:m + 1], y2[m])
                else:
                    dw_dve(u3, dw9_sb[m], dwb_sb[:, m:m + 1], y2[m])

            # w2 (+bn2/pbn folded bias) + residual -> x3p (padded, f32) + bf16 copy
            for mt in range(2):
                xp3 = x3p[mt][:].rearrange("p (h w) -> p h w", w=WP)
                xb3 = x3b[mt][:].rearrange("p (h w) -> p h w", w=WP)
                nc.vector.memset(xp3[:, :, 0:1], 0.0)
                nc.vector.memset(xp3[:, :, 57:58], 0.0)
                nc.vector.memset(xb3[:, :, 0:1], 0.0)
                nc.vector.memset(xb3[:, :, 57:58], 0.0)
                for nt in range(NNT):
                    cs = slice(nt * NT, (nt + 1) * NT)
                    ps = mpool.tile([128, NT], F32, tag="mm", name="mmps2")
                    for kt in range(8):
                        nc.tensor.matmul(ps[:], w2_sb[kt][:, mt * 128:(mt + 1) * 128],
                                         y2[kt][:, cs], start=(kt == 0), stop=(kt == 7))
                    nc.vector.scalar_tensor_tensor(
                        xp3[:, nt * 8:(nt + 1) * 8, 1:57], ps[:], b2r_sb[:, mt:mt + 1],
                        xres[mt][:, cs], op0=Alu.add, op1=Alu.add)
                    nc.scalar.activation(xb3[:, nt * 8:(nt + 1) * 8, 1:57],
                                         xp3[:, nt * 8:(nt + 1) * 8, 1:57], Act.Copy)

            # final blk dwconv -> fT: 8 neighbor taps on PE (bf16 copy) + bias
            # via ones-row matmul; exact-fp32 center/residual fused in the
            # DVE evict: f = psum + (1 + w_center) * x3  (+bkb via matmul).
            for ct in range(2):
                xb3 = x3b[ct][:].rearrange("p (h w) -> p h w", w=WP)
                xp3 = x3p[ct][:].rearrange("p (h w) -> p h w", w=WP)
                diag = build_diag(bk9_sb[ct])
                for nt in range(NNT):
                    ps = dps.tile([128, NT], F32, name="blkps")
                    r0 = nt * 8
                    nc.tensor.matmul(ps[:], bkb_row[0:1, ct * 128:(ct + 1) * 128],
                                     onesn[:], start=True, stop=False)
                    wins = tap_windows(r0)
                    for i, (dy, dx, rlo, rhi) in enumerate(wins):
                        nc.tensor.matmul(
                            ps[:, (rlo - r0) * WS:(rhi - r0) * WS],
                            diag[tap_idx(dy, dx)][:],
                            xb3[:, rlo + dy:rhi + dy, 1 + dx:57 + dx],
                            start=False, stop=(i == len(wins) - 1))
                    fo = accp.tile([128, NT], F32, tag="fout", name="fout", bufs=3)
                    f3 = fo[:].rearrange("p (h w) -> p h w", w=WS)
                    nc.vector.scalar_tensor_tensor(
                        f3[:, :, :], xp3[:, r0:r0 + 8, 1:57], bk9_sb[ct][:, 4:5],
                        ps[:].rearrange("p (h w) -> p h w", w=WS),
                        op0=Alu.mult, op1=Alu.add)
                    nc.sync.dma_start(
                        out=fT_d[ct * 128:(ct + 1) * 128, r0 * WS:(r0 + 8) * WS],
                        in_=fo[:])

    nc.compile()
    return nc


_CACHE = {}


def _get_program():
    if "nc" not in _CACHE:
        _CACHE["nc"] = _build_program()
    return _CACHE["nc"]


def _prep_inputs(inputs):
    f64 = np.float64
    g1 = inputs["ln1_g"].astype(f64); b1ln = inputs["ln1_b"].astype(f64)
    g2 = inputs["ln2_g"].astype(f64); b2ln = inputs["ln2_b"].astype(f64)
    scale = DH ** -0.5

    def bn_ac(g, b, m, v):
        a = np.asarray(g, f64) / np.sqrt(np.asarray(v, f64) + EPS)
        return a, np.asarray(b, f64) - np.asarray(m, f64) * a

    wq = np.asarray(inputs["wq"], f64); wk = np.asarray(inputs["wk"], f64)
    wv = np.asarray(inputs["wv"], f64); wo = np.asarray(inputs["wo"], f64)

    wq_eff = wq * g1[None, :] * scale
    bq_eff = (wq @ b1ln + np.asarray(inputs["bq"], f64)) * scale

    sa, sc = bn_ac(inputs["srbn_g"], inputs["srbn_b"], inputs["srbn_m"], inputs["srbn_v"])
    srw4 = np.asarray(inputs["sr_w"], f64).reshape(C, 4)  # [c, ky*2+kx]
    srw_eff = srw4 * (g1 * sa)[:, None]
    d_const = sa * (b1ln * srw4.sum(1) + np.asarray(inputs["sr_b"], f64)) + sc
    bk_eff = wk @ d_const + np.asarray(inputs["bk"], f64)
    bv_eff = wv @ d_const + np.asarray(inputs["bv"], f64)

    w1 = np.asarray(inputs["w1"], f64)
    w1_eff = w1 * g2[None, :]
    b1_eff = w1 @ b2ln + np.asarray(inputs["b1"], f64)
    a1_, c1_ = bn_ac(inputs["bn1_g"], inputs["bn1_b"], inputs["bn1_m"], inputs["bn1_v"])

    dw9 = np.asarray(inputs["dw_w"], f64).reshape(HID, 9).copy()
    dw9[:, 4] += 1.0  # residual fold
    dwb = np.asarray(inputs["dw_b"], f64)

    pa, pc = bn_ac(inputs["pbn_g"], inputs["pbn_b"], inputs["pbn_m"], inputs["pbn_v"])
    a2_, c2_ = bn_ac(inputs["bn2_g"], inputs["bn2_b"], inputs["bn2_m"], inputs["bn2_v"])
    w2 = np.asarray(inputs["w2"], f64)
    w2_eff = (w2 * pa[None, :]) * a2_[:, None]
    b2_eff = a2_ * (w2 @ pc + np.asarray(inputs["b2"], f64)) + c2_

    bk9 = np.asarray(inputs["blkdw_w"], f64).reshape(C, 9).copy()
    bk9[:, 4] += 1.0
    bkb = np.asarray(inputs["blkdw_b"], f64)

    bf = lambda a: np.ascontiguousarray(np.asarray(a, np.float32)).astype(BF16)
    f32 = lambda a: np.ascontiguousarray(np.asarray(a, np.float32))

    shared = {
        "rpT": np.ascontiguousarray(np.exp(
            np.asarray(inputs["relative_pos"], np.float64)).transpose(0, 2, 1)).astype(BF16),
        "wqT": bf(wq_eff.T), "wkT": bf(wk.T), "wvT": bf(wv.T), "woT": bf(wo.T),
        "w1T": bf(w1_eff.T), "w2T": bf(w2_eff.T),
        "bq": f32(bq_eff), "bk": f32(bk_eff), "bvr": f32(bv_eff[None, :]),
        "bo": f32(inputs["bo"]), "b1": f32(b1_eff), "a1": f32(a1_), "c1": f32(c1_),
        "b2r": f32(b2_eff), "srw": f32(srw_eff), "dw9": f32(dw9), "dwb": f32(dwb),
        "bk9": f32(bk9), "bkb": bf(bkb[None, :]),
        "eyeb": np.eye(128, dtype=np.float32).astype(BF16),
        "eyef": np.eye(128, dtype=np.float32),
        "onesr": np.ones((1, 128), np.float32).astype(BF16),
    }
    x = np.asarray(inputs["x"], np.float32)
    in_maps = []
    for b in range(B):
        m = dict(shared)
        m["xT"] = np.ascontiguousarray(x[b].T)
        in_maps.append(m)
    return in_maps


def kernel(**inputs):
    from concourse.bass_utils import run_bass_kernel_spmd
    nc = _get_program()
    in_maps = _prep_inputs(inputs)
    res = run_bass_kernel_spmd(nc, in_maps, core_ids=list(range(B)))
    out = np.stack([res.results[b]["fT"].T for b in range(B)], axis=0)
    return np.ascontiguousarray(out, dtype=np.float32)


# revision 14
# speedup vs baseline: 40.6548x; 40.6548x over previous
"""Fused PVT-style transformer block kernel for Trainium2 (8 NeuronCores).

Sharding: pure data-parallel over batch B=8 -> one batch item per core.
Layout: channel-major ("transposed") activations [C(part), N(free)] throughout;
host pre-transposes x and relative_pos, post-transposes the output.

Per-core pipeline (N=3136=56x56 tokens, C=256, 4 heads x 64, KV=784=28x28,
HID=1024):
  LN1 (PE ones-matmul stats + PE K=1 broadcast + DVE apply; gamma/beta folded
  into downstream weights) -> q/k/v projections (bf16 PE) with the 2x2/s2
  spatial-reduction depthwise conv on DVE -> flash attention per (head,
  q-tile): scores^T = k^T.T @ q^T with rel-pos bias added via identity matmul
  into PSUM, exp on ACT (no max-subtraction: logits are O(1)), AV matmul with
  ones-row-augmented V giving the softmax denominator for free -> wo
  projection + residual -> LN2 -> conv1x1 (+gelu+bn1) -> 3x3 depthwise conv
  split across PE (fp32r diagonal matmuls into PSUM) and DVE (fused
  scalar_tensor_tensor taps) -> gelu -> conv1x1 (bn2/pbn folded) + residual ->
  final 3x3 depthwise conv (residual folded into center tap) -> output.
"""

import numpy as np
import ml_dtypes

B, N, C, NH, DH, KV, HID = 8, 3136, 256, 4, 64, 784, 1024
HS = WS = 56
NT = 448            # n-tile (8 rows of 56)
NNT = N // NT       # 7
KT = 112            # kv tile
NKT = KV // KT      # 7
EPS = 1e-5
BF16 = ml_dtypes.bfloat16

DW_PE_TILES = (0, 1, 2, 3, 4, 5, 6)   # HID ch-tiles whose dwconv runs on PE
BLK_PE_TILES = (0,)             # blk dwconv ch-tiles on PE

TAPS = [(dy, dx) for dy in (-1, 0, 1) for dx in (-1, 0, 1) if (dy, dx) != (0, 0)]


def tap_idx(dy, dx):
    return (dy + 1) * 3 + (dx + 1)


def _build_program(iters=1):
    import concourse.bacc as bacc
    import concourse.mybir as mybir
    import concourse.tile as tile
    from contextlib import ExitStack

    dt = mybir.dt
    F32, BF, F32R = dt.float32, dt.bfloat16, dt.float32r
    Alu = mybir.AluOpType
    Act = mybir.ActivationFunctionType

    nc = bacc.Bacc("TRN2", target_bir_lowering=False, debug=False, num_devices=8)

    def din(name, shape, dtype):
        return nc.dram_tensor(name, shape, dtype, kind="ExternalInput")

    xT_d = din("xT", [C, N], F32)
    rpT_d = din("rpT", [NH, KV, N], BF)
    wqT_d = din("wqT", [C, C], BF)
    wkT_d = din("wkT", [C, C], BF)
    wvT_d = din("wvT", [C, C], BF)
    woT_d = din("woT", [C, C], BF)
    w1T_d = din("w1T", [C, HID], BF)
    w2T_d = din("w2T", [HID, C], BF)
    bq_d = din("bq", [C], F32)
    bk_d = din("bk", [C], F32)
    bvr_d = din("bvr", [1, C], F32)
    bo_d = din("bo", [C], F32)
    b1_d = din("b1", [HID], F32)
    a1_d = din("a1", [HID], F32)
    c1_d = din("c1", [HID], F32)
    b2r_d = din("b2r", [C], F32)
    srw_d = din("srw", [C, 4], F32)
    dw9_d = din("dw9", [HID, 9], F32)
    dwb_d = din("dwb", [HID], F32)
    bk9_d = din("bk9", [C, 9], F32)
    bkb_d = din("bkb", [1, C], BF)
    eyeb_d = din("eyeb", [128, 128], BF)
    eyef_d = din("eyef", [128, 128], F32)
    onesr_d = din("onesr", [1, 128], BF)
    fT_d = nc.dram_tensor("fT", [C, N], F32, kind="ExternalOutput")

    def r32(ap):
        return ap.bitcast(F32R)

    with tile.TileContext(nc) as tc, ExitStack() as octx:
        wpool = octx.enter_context(tc.tile_pool(name="weights", bufs=1))
        persist = octx.enter_context(tc.tile_pool(name="persist", bufs=1))

        def wload(dram_ap, shape, dtype, tag):
            t = wpool.tile(shape, dtype, tag=tag, name=tag)
            nc.sync.dma_start(out=t[:], in_=dram_ap)
            return t

        wq_sb = [wload(wqT_d[k * 128:(k + 1) * 128, :], [128, C], BF, f"wq{k}") for k in range(2)]
        wk_sb = [wload(wkT_d[k * 128:(k + 1) * 128, :], [128, C], BF, f"wk{k}") for k in range(2)]
        wv_sb = [wload(wvT_d[k * 128:(k + 1) * 128, :], [128, C], BF, f"wv{k}") for k in range(2)]
        wo_sb = [wload(woT_d[k * 128:(k + 1) * 128, :], [128, C], BF, f"wo{k}") for k in range(2)]
        w1_sb = [wload(w1T_d[k * 128:(k + 1) * 128, :], [128, HID], BF, f"w1{k}") for k in range(2)]
        w2_sb = [wload(w2T_d[k * 128:(k + 1) * 128, :], [128, C], BF, f"w2{k}") for k in range(8)]
        eyeb = wload(eyeb_d[:, :], [128, 128], BF, "eyeb")
        eyef = wload(eyef_d[:, :], [128, 128], F32, "eyef")
        onesr = wload(onesr_d[:, :], [1, 128], BF, "onesr")
        srw_sb = [wload(srw_d.ap().rearrange("(t p) k -> p t k", p=128)[:, t, :],
                        [128, 4], F32, f"srw{t}") for t in range(2)]
        dw9_sb = [wload(dw9_d.ap().rearrange("(t p) k -> p t k", p=128)[:, t, :],
                        [128, 9], F32, f"dw9_{t}") for t in range(8)]
        bk9_sb = [wload(bk9_d.ap().rearrange("(t p) k -> p t k", p=128)[:, t, :],
                        [128, 9], F32, f"bk9_{t}") for t in range(2)]

        def vload(dram, n, tag):
            t = wpool.tile([128, n // 128], F32, tag=tag, name=tag)
            nc.sync.dma_start(out=t[:], in_=dram.ap().rearrange("(t p) -> p t", p=128))
            return t

        bq_sb = vload(bq_d, C, "bq")
        bk_sb = vload(bk_d, C, "bk")
        bo_sb = vload(bo_d, C, "bo")
        b1_sb = vload(b1_d, HID, "b1")
        a1_sb = vload(a1_d, HID, "a1")
        c1_sb = vload(c1_d, HID, "c1")
        b2r_sb = vload(b2r_d, C, "b2r")
        dwb_sb = vload(dwb_d, HID, "dwb")
        bvr_bc = wpool.tile([128, C], F32, tag="bvr")
        nc.sync.dma_start(out=bvr_bc[:], in_=bvr_d.ap().partition_broadcast(128))
        bkb_row = wload(bkb_d[:, :], [1, C], BF, "bkb")
        onesn = wpool.tile([1, NT], BF, tag="onesn")
        nc.vector.memset(onesn[:], 1.0)
        onescol = wpool.tile([128, 1], F32, tag="onescol")
        nc.vector.memset(onescol[:], 1.0)
        onescol_b = wpool.tile([128, 1], BF, tag="onescol_b")
        nc.vector.memset(onescol_b[:], 1.0)
        epscol = wpool.tile([128, 1], F32, tag="epscol")
        nc.vector.memset(epscol[:], EPS)

        # residual stream (f32), reused x -> x+attn -> x+mlp in place
        xres = [persist.tile([128, N], F32, tag=f"xres{t}", name=f"xres{t}") for t in range(2)]
        # LN output (bf16, un-affine'd), reused for LN1 and LN2
        hbuf = [persist.tile([128, N], BF, tag=f"hbuf{t}", name=f"hbuf{t}") for t in range(2)]

        def body(suffix):
            for t in range(2):
                for nt in range(NNT):
                    cs = slice(nt * NT, (nt + 1) * NT)
                    nc.sync.dma_start(out=xres[t][:, cs], in_=xT_d[t * 128:(t + 1) * 128, cs])
            run_stages(suffix)

        def layer_norm(suffix):
            """hbuf <- (xres - mean_c) * rsqrt(var_c + eps)."""
            with ExitStack() as ctx:
                sqp = ctx.enter_context(tc.tile_pool(name=f"ln_sq{suffix}", bufs=3))
                stp = ctx.enter_context(tc.tile_pool(name=f"ln_st{suffix}", bufs=2, space="PSUM"))
                bcp = ctx.enter_context(tc.tile_pool(name=f"ln_bc{suffix}", bufs=2, space="PSUM"))
                rowp = ctx.enter_context(tc.tile_pool(name=f"ln_row{suffix}", bufs=2))
                tmpp = ctx.enter_context(tc.tile_pool(name=f"ln_tmp{suffix}", bufs=3))
                for nt in range(NNT):
                    cs = slice(nt * NT, (nt + 1) * NT)
                    st0 = stp.tile([1, NT], F32, tag="st0")
                    for ct in range(2):
                        nc.tensor.matmul(st0[:], onescol[:], xres[ct][:, cs],
                                         start=(ct == 0), stop=(ct == 1))
                    st1 = stp.tile([1, NT], F32, tag="st1")
                    for ct in range(2):
                        sq = sqp.tile([128, NT], BF)
                        nc.scalar.square(sq[:], xres[ct][:, cs])
                        nc.tensor.matmul(st1[:], onescol_b[:], sq[:],
                                         start=(ct == 0), stop=(ct == 1))
                    m2 = rowp.tile([1, NT], F32, tag="m2")
                    nc.scalar.activation(m2[:], st0[:], Act.Square, scale=1.0 / C)
                    var = rowp.tile([1, NT], F32, tag="var")
                    nc.vector.scalar_tensor_tensor(var[:], st1[:], 1.0 / C, m2[:],
                                                   op0=Alu.mult, op1=Alu.subtract)
                    sd = rowp.tile([1, NT], F32, tag="sd")
                    nc.scalar.activation(sd[:], var[:], Act.Sqrt, bias=epscol[0:1, :])
                    arow = rowp.tile([1, NT], BF, tag="arow")
                    with nc.allow_low_precision("bf16 rstd broadcast row"):
                        nc.vector.reciprocal(arow[:], sd[:])
                    crow = rowp.tile([1, NT], BF, tag="crow")
                    nc.vector.scalar_tensor_tensor(crow[:], st0[:], -1.0 / C, arow[:],
                                                   op0=Alu.mult, op1=Alu.mult)
                    aps = bcp.tile([128, NT], F32, tag="abc")
                    nc.tensor.matmul(aps[:], onesr[:], arow[:])
                    cps = bcp.tile([128, NT], F32, tag="abc")
                    nc.tensor.matmul(cps[:], onesr[:], crow[:])
                    for ct in range(2):
                        t0 = tmpp.tile([128, NT], F32)
                        nc.vector.tensor_mul(t0[:], xres[ct][:, cs], aps[:])
                        nc.vector.tensor_add(hbuf[ct][:, cs], t0[:], cps[:])

        def run_stages(it):
            run_stage1(it)
            run_stage2(it)

        # ================= stage 1: LN1 + attention =================
        def run_stage1(it):
            ctx = ExitStack()
            layer_norm("1" + it)
            apool = ctx.enter_context(tc.tile_pool(name="attn_sb", bufs=1))
            cT = [apool.tile([128, KV], BF, tag=f"cT{t}", name=f"cT{t}") for t in range(2)]
            k_sb = [apool.tile([128, KV], BF, tag=f"k{t}", name=f"k{t}") for t in range(2)]
            v_sb = apool.tile([128, NKT * 260], BF, tag="v", name="v_sb")
            q_sb = [apool.tile([128, N], BF, tag=f"q{t}", name=f"q{t}") for t in range(2)]
            o_cat = [apool.tile([128, N], BF, tag=f"ocat{t}", name=f"ocat{t}") for t in range(2)]

            with ExitStack() as pctx:
                mmp = pctx.enter_context(tc.tile_pool(name="proj_ps", bufs=3, space="PSUM"))
                # SR 2x2/s2 depthwise conv on hbuf -> cT [C, 784] bf16
                for ct in range(2):
                    h4 = hbuf[ct][:].rearrange("p (h a w b) -> p h a w b", a=2, b=2, h=28, w=28)
                    c3 = cT[ct][:].rearrange("p (h w) -> p h w", w=28)
                    nc.vector.tensor_scalar_mul(c3[:, :, :], h4[:, :, 0, :, 0], srw_sb[ct][:, 0:1])
                    for ky, kx in ((0, 1), (1, 0), (1, 1)):
                        ti = ky * 2 + kx
                        nc.vector.scalar_tensor_tensor(
                            c3[:, :, :], h4[:, :, ky, :, kx], srw_sb[ct][:, ti:ti + 1],
                            c3[:, :, :], op0=Alu.mult, op1=Alu.add)

                # k^T = wk @ cT + bk -> [256, 784] bf16
                for mt in range(2):
                    for n0, nsz in ((0, 448), (448, 336)):
                        ps = mmp.tile([128, NT], F32, tag="mm")
                        for kt in range(2):
                            nc.tensor.matmul(ps[:, :nsz], wk_sb[kt][:, mt * 128:(mt + 1) * 128],
                                             cT[kt][:, n0:n0 + nsz], start=(kt == 0), stop=(kt == 1))
                        nc.scalar.activation(k_sb[mt][:, n0:n0 + nsz], ps[:, :nsz], Act.Identity,
                                             bias=bk_sb[:, mt:mt + 1])

                # v (+ones col per head) -> v_sb [112, 7*260] bf16
                for kt in range(NKT):
                    ps = mmp.tile([128, NT], F32, tag="mm")
                    for ct in range(2):
                        nc.tensor.matmul(ps[0:KT, 0:C], cT[ct][:, kt * KT:(kt + 1) * KT],
                                         wv_sb[ct][:], start=(ct == 0), stop=(ct == 1))
                    for h in range(NH):
                        nc.vector.scalar_tensor_tensor(
                            v_sb[0:KT, kt * 260 + h * 65: kt * 260 + h * 65 + 64],
                            ps[0:KT, h * 64:(h + 1) * 64], 1.0,
                            bvr_bc[0:KT, h * 64:(h + 1) * 64], op0=Alu.mult, op1=Alu.add)
                        nc.vector.memset(v_sb[0:KT, kt * 260 + h * 65 + 64: kt * 260 + h * 65 + 65], 1.0)

                # q^T = wq @ hbuf + bq -> [256, 3136] bf16
                for mt in range(2):
                    for nt in range(NNT):
                        cs = slice(nt * NT, (nt + 1) * NT)
                        ps = mmp.tile([128, NT], F32, tag="mm")
                        for kt in range(2):
                            nc.tensor.matmul(ps[:], wq_sb[kt][:, mt * 128:(mt + 1) * 128],
                                             hbuf[kt][:, cs], start=(kt == 0), stop=(kt == 1))
                        nc.scalar.activation(q_sb[mt][:, cs], ps[:], Act.Identity,
                                             bias=bq_sb[:, mt:mt + 1])

            # flash attention (heads interleaved for PE row-group packing;
            # rel-pos bias applied as exp(s)*exp(rp) with host-precomputed
            # exp(rp) multiplied in on DVE)
            with ExitStack() as pctx:
                rpp = pctx.enter_context(tc.tile_pool(name="rp", bufs=2))
                ppp = pctx.enter_context(tc.tile_pool(name="pexp", bufs=2))
                sps = pctx.enter_context(tc.tile_pool(name="spsum", bufs=3, space="PSUM"))
                ops = pctx.enter_context(tc.tile_pool(name="opsum", bufs=2, space="PSUM"))
                rps = pctx.enter_context(tc.tile_pool(name="rpsum", bufs=1, space="PSUM"))
                rsp = pctx.enter_context(tc.tile_pool(name="rsb", bufs=2))
                for qt in range(NNT):
                    cs = slice(qt * NT, (qt + 1) * NT)
                    for h in range(NH):
                        ht, hr = h // 2, (h % 2) * 64
                        rp_t = rpp.tile([KT, NKT, NT], BF, name="rp_t")
                        nc.sync.dma_start(
                            out=rp_t[:],
                            in_=rpT_d.ap()[h].rearrange("(t p) n -> p t n", p=KT)[:, :, cs])
                        p_t = ppp.tile([KT, NKT, NT], BF, name="p_t")
                        for kt in range(NKT):
                            s_ps = sps.tile([KT, NT], F32, name="s_ps")
                            nc.tensor.matmul(s_ps[:], k_sb[ht][hr:hr + 64, kt * KT:(kt + 1) * KT],
                                             q_sb[ht][hr:hr + 64, cs], start=True, stop=True)
                            et = rsp.tile([KT, NT], BF, tag="et", name="et", bufs=3)
                            nc.scalar.activation(et[:], s_ps[:], Act.Exp)
                            nc.vector.tensor_mul(p_t[:, kt, :], et[:], rp_t[:, kt, :])
                        o_ps = ops.tile([65, NT], F32, name="o_ps")
                        for kt in range(NKT):
                            nc.tensor.matmul(o_ps[:],
                                             v_sb[0:KT, kt * 260 + h * 65: kt * 260 + (h + 1) * 65],
                                             p_t[:, kt, :], start=(kt == 0), stop=(kt == NKT - 1))
                        rrow = rsp.tile([1, NT], BF, tag="rrow", name="rrow")
                        with nc.allow_low_precision("bf16 softmax denom row"):
                            nc.vector.reciprocal(rrow[:], o_ps[64:65, :])
                        rb_ps = rps.tile([64, NT], F32, name="rb_ps")
                        nc.tensor.matmul(rb_ps[:], onesr[0:1, 0:64], rrow[:])
                        rb_sb = rsp.tile([64, NT], F32, tag="rbsb", name="rb_sb")
                        nc.vector.tensor_copy(rb_sb[:], rb_ps[:])
                        nc.vector.tensor_mul(o_cat[ht][hr:hr + 64, cs], o_ps[0:64, :], rb_sb[:])

            # wo projection + residual into xres (in place)
            with ExitStack() as pctx:
                mmp = pctx.enter_context(tc.tile_pool(name="wo_ps", bufs=3, space="PSUM"))
                for nt in range(NNT):
                    for mt in range(2):
                        cs = slice(nt * NT, (nt + 1) * NT)
                        ps = mmp.tile([128, NT], F32, tag="mm")
                        for kt in range(2):
                            nc.tensor.matmul(ps[:], wo_sb[kt][:, mt * 128:(mt + 1) * 128],
                                             o_cat[kt][:, cs], start=(kt == 0), stop=(kt == 1))
                        nc.vector.scalar_tensor_tensor(xres[mt][:, cs], ps[:], bo_sb[:, mt:mt + 1],
                                                       xres[mt][:, cs], op0=Alu.add, op1=Alu.add)

            ctx.close()

        # ================= stage 2: LN2 + conv-MLP + blk dwconv =================
        # dwconv inputs are x-padded to width 58 (zero cols 0 and 57) so all
        # taps are full-width and matmul outputs stay flat 2D.
        WP = WS + 2

        def run_stage2(it):
            ctx = ExitStack()
            layer_norm("2" + it)
            mpool = ctx.enter_context(tc.tile_pool(name="mlp_ps", bufs=3, space="PSUM"))
            dps = ctx.enter_context(tc.tile_pool(name="dw_ps", bufs=2, space="PSUM"))
            upool = ctx.enter_context(tc.tile_pool(name="u", bufs=2))
            accp = ctx.enter_context(tc.tile_pool(name="dwacc", bufs=2))
            digp = ctx.enter_context(tc.tile_pool(name="diag", bufs=2))
            y2p = ctx.enter_context(tc.tile_pool(name="y2", bufs=1))
            y2 = [y2p.tile([128, N], BF, tag=f"y2_{m}", name=f"y2_{m}") for m in range(8)]
            x3p = [y2p.tile([128, HS * WP], F32, tag=f"x3p{t}", name=f"x3p{t}")
                   for t in range(2)]
            x3b = [y2p.tile([128, HS * WP], BF, tag=f"x3b{t}", name=f"x3b{t}")
                   for t in range(2)]

            def build_diag(w9_sb):
                diag = []
                for t in range(9):
                    dg = digp.tile([128, 128], BF, tag=f"dg{t}", name=f"dg{t}")
                    nc.vector.tensor_scalar_mul(dg[:], eyeb[:], w9_sb[:, t:t + 1])
                    diag.append(dg)
                return diag

            def tap_windows(r0):
                wins = []
                for dy, dx in TAPS:
                    rlo = max(r0, 1 if dy < 0 else 0)
                    rhi = min(r0 + 8, HS - (1 if dy > 0 else 0))
                    if rlo < rhi:
                        wins.append((dy, dx, rlo, rhi))
                return wins

            def dw_pe(src3, w9_sb, bias_col, dst):
                """3x3 depthwise conv of padded bf16 src3 [128,56,58] via PE
                diag matmuls (all 9 taps); gelu evict with bias -> dst bf16."""
                diag = build_diag(w9_sb)
                for nt in range(NNT):
                    ps = dps.tile([128, NT], F32, name="dwps")
                    r0 = nt * 8
                    nc.tensor.matmul(ps[:], diag[4][:], src3[:, r0:r0 + 8, 1:57],
                                     start=True, stop=False)
                    wins = tap_windows(r0)
                    for i, (dy, dx, rlo, rhi) in enumerate(wins):
                        nc.tensor.matmul(
                            ps[:, (rlo - r0) * WS:(rhi - r0) * WS],
                            diag[tap_idx(dy, dx)][:],
                            src3[:, rlo + dy:rhi + dy, 1 + dx:57 + dx],
                            start=False, stop=(i == len(wins) - 1))
                    nc.scalar.activation(dst[:, r0 * WS:(r0 + 8) * WS], ps[:], Act.Gelu,
                                         bias=bias_col)

            def dw_dve(src3, w9_sb, bias_col, dst):
                """3x3 depthwise conv on DVE: center-tap init (+bias), 8 stt taps."""
                acc = accp.tile([128, N], F32, name="acc")
                a3 = acc[:].rearrange("p (h w) -> p h w", w=WS)
                nc.vector.tensor_scalar(a3[:, :, :], src3[:, :, 1:57], w9_sb[:, 4:5],
                                        bias_col, op0=Alu.mult, op1=Alu.add)
                for dy, dx in TAPS:
                    rlo = 1 if dy < 0 else 0
                    rhi = HS - (1 if dy > 0 else 0)
                    t = tap_idx(dy, dx)
                    nc.vector.scalar_tensor_tensor(
                        a3[:, rlo:rhi, :], src3[:, rlo + dy:rhi + dy, 1 + dx:57 + dx],
                        w9_sb[:, t:t + 1], a3[:, rlo:rhi, :], op0=Alu.mult, op1=Alu.add)
                nc.scalar.activation(dst[:], acc[:], Act.Gelu)

            for m in range(8):
                u = upool.tile([128, HS * WP], BF, name="u")
                u3 = u[:].rearrange("p (h w) -> p h w", w=WP)
                nc.vector.memset(u3[:, :, 0:1], 0.0)
                nc.vector.memset(u3[:, :, 57:58], 0.0)
                for nt in range(NNT):
                    cs = slice(nt * NT, (nt + 1) * NT)
                    ps = mpool.tile([128, NT], F32, tag="mm", name="mmps")
                    for kt in range(2):
                        nc.tensor.matmul(ps[:], w1_sb[kt][:, m * 128:(m + 1) * 128],
                                         hbuf[kt][:, cs], start=(kt == 0), stop=(kt == 1))
                    nc.scalar.activation(u3[:, nt * 8:(nt + 1) * 8, 1:57], ps[:], Act.Gelu,
                                         bias=b1_sb[:, m:m + 1])
                nc.vector.tensor_scalar(u3[:, :, 1:57], u3[:, :, 1:57], a1_sb[:, m:m + 1],
                                        c1_sb[:, m:m + 1], op0=Alu.mult, op1=Alu.add)
                if m in DW_PE_TILES:
                    dw_pe(u3, dw9_sb[m], dwb_sb[:, m:m + 1], y2[m])
                else:
                    dw_dve(u3, dw9_sb[m], dwb_sb[:, m:m + 1], y2[m])

            # w2 (+bn2/pbn folded bias) + residual -> x3p (padded, f32) + bf16 copy
            for mt in range(2):
                xp3 = x3p[mt][:].rearrange("p (h w) -> p h w", w=WP)
                xb3 = x3b[mt][:].rearrange("p (h w) -> p h w", w=WP)
                nc.vector.memset(xp3[:, :, 0:1], 0.0)
                nc.vector.memset(xp3[:, :, 57:58], 0.0)
                nc.vector.memset(xb3[:, :, 0:1], 0.0)
                nc.vector.memset(xb3[:, :, 57:58], 0.0)
            for nt in range(NNT):
                for mt in range(2):
                    xp3 = x3p[mt][:].rearrange("p (h w) -> p h w", w=WP)
                    xb3 = x3b[mt][:].rearrange("p (h w) -> p h w", w=WP)
                    cs = slice(nt * NT, (nt + 1) * NT)
                    ps = mpool.tile([128, NT], F32, tag="mm", name="mmps2")
                    for kt in range(8):
                        nc.tensor.matmul(ps[:], w2_sb[kt][:, mt * 128:(mt + 1) * 128],
                                         y2[kt][:, cs], start=(kt == 0), stop=(kt == 7))
                    nc.vector.scalar_tensor_tensor(
                        xp3[:, nt * 8:(nt + 1) * 8, 1:57], ps[:], b2r_sb[:, mt:mt + 1],
                        xres[mt][:, cs], op0=Alu.add, op1=Alu.add)
                    nc.scalar.activation(xb3[:, nt * 8:(nt + 1) * 8, 1:57],
                                         xp3[:, nt * 8:(nt + 1) * 8, 1:57], Act.Copy)

            # final blk dwconv -> fT: 8 neighbor taps on PE (bf16 copy) + bias
            # via ones-row matmul; exact-fp32 center/residual fused in the
            # DVE evict: f = psum + (1 + w_center) * x3  (+bkb via matmul).
            for ct in range(2):
                xb3 = x3b[ct][:].rearrange("p (h w) -> p h w", w=WP)
                xp3 = x3p[ct][:].rearrange("p (h w) -> p h w", w=WP)
                diag = build_diag(bk9_sb[ct])
                for nt in range(NNT):
                    ps = dps.tile([128, NT], F32, name="blkps")
                    r0 = nt * 8
                    nc.tensor.matmul(ps[:], bkb_row[0:1, ct * 128:(ct + 1) * 128],
                                     onesn[:], start=True, stop=False)
                    wins = tap_windows(r0)
                    for i, (dy, dx, rlo, rhi) in enumerate(wins):
                        nc.tensor.matmul(
                            ps[:, (rlo - r0) * WS:(rhi - r0) * WS],
                            diag[tap_idx(dy, dx)][:],
                            xb3[:, rlo + dy:rhi + dy, 1 + dx:57 + dx],
                            start=False, stop=(i == len(wins) - 1))
                    fo = accp.tile([128, NT], F32, tag="fout", name="fout", bufs=3)
                    f3 = fo[:].rearrange("p (h w) -> p h w", w=WS)
                    nc.vector.scalar_tensor_tensor(
                        f3[:, :, :], xp3[:, r0:r0 + 8, 1:57], bk9_sb[ct][:, 4:5],
                        ps[:].rearrange("p (h w) -> p h w", w=WS),
                        op0=Alu.mult, op1=Alu.add)
                    nc.sync.dma_start(
                        out=fT_d[ct * 128:(ct + 1) * 128, r0 * WS:(r0 + 8) * WS],
                        in_=fo[:])
            ctx.close()

        for it in range(iters):
            body(f"_i{it}")

    nc.compile()
    return nc


_CACHE = {}


def _get_program():
    if "nc" not in _CACHE:
        _CACHE["nc"] = _build_program()
    return _CACHE["nc"]


def _prep_inputs(inputs):
    f64 = np.float64
    g1 = inputs["ln1_g"].astype(f64); b1ln = inputs["ln1_b"].astype(f64)
    g2 = inputs["ln2_g"].astype(f64); b2ln = inputs["ln2_b"].astype(f64)
    scale = DH ** -0.5

    def bn_ac(g, b, m, v):
        a = np.asarray(g, f64) / np.sqrt(np.asarray(v, f64) + EPS)
        return a, np.asarray(b, f64) - np.asarray(m, f64) * a

    wq = np.asarray(inputs["wq"], f64); wk = np.asarray(inputs["wk"], f64)
    wv = np.asarray(inputs["wv"], f64); wo = np.asarray(inputs["wo"], f64)

    wq_eff = wq * g1[None, :] * scale
    bq_eff = (wq @ b1ln + np.asarray(inputs["bq"], f64)) * scale

    sa, sc = bn_ac(inputs["srbn_g"], inputs["srbn_b"], inputs["srbn_m"], inputs["srbn_v"])
    srw4 = np.asarray(inputs["sr_w"], f64).reshape(C, 4)  # [c, ky*2+kx]
    srw_eff = srw4 * (g1 * sa)[:, None]
    d_const = sa * (b1ln * srw4.sum(1) + np.asarray(inputs["sr_b"], f64)) + sc
    bk_eff = wk @ d_const + np.asarray(inputs["bk"], f64)
    bv_eff = wv @ d_const + np.asarray(inputs["bv"], f64)

    w1 = np.asarray(inputs["w1"], f64)
    w1_eff = w1 * g2[None, :]
    b1_eff = w1 @ b2ln + np.asarray(inputs["b1"], f64)
    a1_, c1_ = bn_ac(inputs["bn1_g"], inputs["bn1_b"], inputs["bn1_m"], inputs["bn1_v"])

    dw9 = np.asarray(inputs["dw_w"], f64).reshape(HID, 9).copy()
    dw9[:, 4] += 1.0  # residual fold
    dwb = np.asarray(inputs["dw_b"], f64)

    pa, pc = bn_ac(inputs["pbn_g"], inputs["pbn_b"], inputs["pbn_m"], inputs["pbn_v"])
    a2_, c2_ = bn_ac(inputs["bn2_g"], inputs["bn2_b"], inputs["bn2_m"], inputs["bn2_v"])
    w2 = np.asarray(inputs["w2"], f64)
    w2_eff = (w2 * pa[None, :]) * a2_[:, None]
    b2_eff = a2_ * (w2 @ pc + np.asarray(inputs["b2"], f64)) + c2_

    bk9 = np.asarray(inputs["blkdw_w"], f64).reshape(C, 9).copy()
    bk9[:, 4] += 1.0
    bkb = np.asarray(inputs["blkdw_b"], f64)

    bf = lambda a: np.ascontiguousarray(np.asarray(a, np.float32)).astype(BF16)
    f32 = lambda a: np.ascontiguousarray(np.asarray(a, np.float32))

    shared = {
        "rpT": np.ascontiguousarray(np.exp(
            np.asarray(inputs["relative_pos"], np.float64)).transpose(0, 2, 1)).astype(BF16),
        "wqT": bf(wq_eff.T), "wkT": bf(wk.T), "wvT": bf(wv.T), "woT": bf(wo.T),
        "w1T": bf(w1_eff.T), "w2T": bf(w2_eff.T),
        "bq": f32(bq_eff), "bk": f32(bk_eff), "bvr": f32(bv_eff[None, :]),
        "bo": f32(inputs["bo"]), "b1": f32(b1_eff), "a1": f32(a1_), "c1": f32(c1_),
        "b2r": f32(b2_eff), "srw": f32(srw_eff), "dw9": f32(dw9), "dwb": f32(dwb),
        "bk9": f32(bk9), "bkb": bf(bkb[None, :]),
        "eyeb": np.eye(128, dtype=np.float32).astype(BF16),
        "eyef": np.eye(128, dtype=np.float32),
        "onesr": np.ones((1, 128), np.float32).astype(BF16),
    }
    x = np.asarray(inputs["x"], np.float32)
    in_maps = []
    for b in range(B):
        m = dict(shared)
        m["xT"] = np.ascontiguousarray(x[b].T)
        in_maps.append(m)
    return in_maps


def kernel(**inputs):
    from concourse.bass_utils import run_bass_kernel_spmd
    nc = _get_program()
    in_maps = _prep_inputs(inputs)
    res = run_bass_kernel_spmd(nc, in_maps, core_ids=list(range(B)))
    out = np.stack([res.results[b]["fT"].T for b in range(B)], axis=0)
    return np.ascontiguousarray(out, dtype=np.float32)
